# revision 16
# baseline (speedup 1.0000x reference)
"""Trainium2 Bass kernel for nn_AuxiliaryGIN (2-layer GIN + 4 output heads).

Sharding (8 NeuronCores): nodes row-partitioned 6250/core (edges partitioned
by destination node so scatter-adds stay local); 128x128 weights replicated;
halo = full feature table in HBM per layer (layer 2's is assembled with an
AllGather).

Per core, per layer:
  * edges are grouped by destination block (128 nodes) and split into
    lo/hi source tables (dma_gather indices are int16, so a feature table is
    limited to 32768 rows)
  * batched dma_gather pulls x[src] rows into SBUF tiles G [128 edges, 128 d]
  * a one-hot S^T [128 edges, 128 dst] per tile is built on the vector engine
    with a broadcast is_equal against an iota row
  * agg^T[d, dst] = sum_tiles G^T @ S^T accumulates in PSUM; the group is
    seeded with I^T @ x^T to fold in GIN's "+x" term
  * the GIN MLP runs channel-major: W is the natural lhsT, BN/bias/ReLU are
    per-partition scalar-engine activations
  * layer 1 output is PE-transposed to row-major and AllGathered into the
    shared feature table for layer 2's gathers
  * heads: per node block, one matmul against the concatenated head weights,
    then log_softmax / softmax / sigmoid along the free axis
"""

import math
import os

import numpy as np

_DBG = set(os.environ.get("KDBG", "").split(",")) - {""}

import concourse.bacc as bacc
import concourse.bass as bass
import concourse.mybir as mybir
import concourse.tile as tile
from concourse import bass_utils
from concourse.masks import make_identity

P = 128
N_CORES = 8
D = 128
N_HEAD = 82  # 40 cls + 40 sim + 1 hom + 1 ent
SPLIT = 32768  # dma_gather int16 index limit per table
NBLK_SC = 4  # node blocks per superchunk (gather/MLP granularity)
BN_EPS = 1e-5

f32 = mybir.dt.float32
i16 = mybir.dt.int16

_CACHE: dict = {}


# --------------------------------------------------------------------------
# host-side planning
# --------------------------------------------------------------------------

def _plan_layer(src_rows, dst_local_all, core_of_dst, cfg):
    """Group each core's edges by destination block, split lo/hi by source
    row, and pad per-(block, table) tile counts to the max over cores so the
    SPMD program is shape-uniform.

    src_rows: per-edge row index into the (possibly padded) feature table.
    dst_local_all: per-edge destination index local to its core [0, npc).
    core_of_dst: per-edge owning core.

    Returns (plan, idx16 [n_cores, 128, TT*8], dstloc [n_cores, 128, TT]).
    plan is a list of superchunks:
      {"t0": int, "nA": int, "nB": int,
       "blocks": [(kb, a0, ta, b0, tb), ...]}  # tile offsets local to sc
    """
    nc_, nb, split = cfg["n_cores"], cfg["nb"], cfg["split"]

    # bucket edges
    lo_rows = [[None] * nb for _ in range(nc_)]
    lo_dloc = [[None] * nb for _ in range(nc_)]
    hi_rows = [[None] * nb for _ in range(nc_)]
    hi_dloc = [[None] * nb for _ in range(nc_)]
    blk_all = dst_local_all >> 7
    dloc_all = dst_local_all & 127
    is_hi = src_rows >= split
    for c in range(nc_):
        cm = core_of_dst == c
        for b in range(nb):
            m = cm & (blk_all == b)
            ml = m & ~is_hi
            mh = m & is_hi
            lo_rows[c][b] = src_rows[ml]
            lo_dloc[c][b] = dloc_all[ml]
            hi_rows[c][b] = src_rows[mh] - split
            hi_dloc[c][b] = dloc_all[mh]

    ta = [max(-(-len(lo_rows[c][b]) // P) for c in range(nc_))
          for b in range(nb)]
    tb = [max(-(-len(hi_rows[c][b]) // P) for c in range(nc_))
          for b in range(nb)]

    # global tile order: per superchunk, all A tiles (block-major) then all B
    plan = []
    t0 = 0
    for s0 in range(0, nb, NBLK_SC):
        blocks = list(range(s0, min(s0 + NBLK_SC, nb)))
        nA = sum(ta[b] for b in blocks)
        nB = sum(tb[b] for b in blocks)
        binfo = []
        a_off, b_off = 0, nA
        for kb, b in enumerate(blocks):
            binfo.append((kb, a_off, ta[b], b_off, tb[b]))
            a_off += ta[b]
            b_off += tb[b]
        plan.append({"t0": t0, "nA": nA, "nB": nB, "blocks": binfo,
                     "blk0": s0, "nblk": len(blocks)})
        t0 += nA + nB
    tt = t0

    idx16 = np.zeros((nc_, P, tt * 8), np.int16)
    dstloc = np.full((nc_, P, tt), -1.0, np.float32)

    def fill(c, tile0, ntiles, rows, dloc):
        if ntiles == 0:
            return
        n_pad = ntiles * P
        r = np.zeros(n_pad, np.int64)
        r[:len(rows)] = rows
        d_ = np.full(n_pad, -1.0, np.float32)
        d_[:len(dloc)] = dloc
        # gather wrapped layout: entry i -> [i % 16, col0 + i // 16]
        idx16[c, :16, tile0 * 8:(tile0 + ntiles) * 8] = \
            r.astype(np.int16).reshape(-1, 16).T
        # dstloc: col t, partition p = entry t*128+p
        dstloc[c, :, tile0:tile0 + ntiles] = d_.reshape(-1, P).T

    for c in range(nc_):
        for sc in plan:
            for kb, a0, ta_, b0, tb_ in sc["blocks"]:
                b = sc["blk0"] + kb
                fill(c, sc["t0"] + a0, ta_, lo_rows[c][b], lo_dloc[c][b])
                fill(c, sc["t0"] + b0, tb_, hi_rows[c][b], hi_dloc[c][b])
    # q7 cores read idx partitions in groups of 16, replicated 8x
    idx16[:, 16:, :] = np.tile(idx16[:, :16, :], (1, 7, 1))
    return plan, idx16, dstloc, tt


# --------------------------------------------------------------------------
# device program
# --------------------------------------------------------------------------

GMAX = 8  # dma_gather hangs the DMA unit above 1024 indices (measured)


def _emit_gather(nc, g_sb, col0, ntiles, table, idx_sb, tile0):
    if ntiles == 0 or "nogather" in _DBG:
        return
    for c0 in range(0, ntiles, GMAX):
        nt = min(GMAX, ntiles - c0)
        n = nt * P
        nc.gpsimd.dma_gather(
            out_ap=g_sb[:, (col0 + c0) * P:(col0 + c0 + nt) * P].rearrange(
                "p (t e) -> p t e", e=D),
            in_ap=table,
            idxs_ap=idx_sb[:, (tile0 + c0) * 8:(tile0 + c0 + nt) * 8],
            num_idxs=n,
            num_idxs_reg=n,
            elem_size=D,
        )


def _build_program(cfg, plans):
    n_nodes, npad = cfg["n_nodes"], cfg["npad"]
    nb, split = cfg["nb"], cfg["split"]
    tts = [cfg["tt1"], cfg["tt2"]]
    ncr = cfg["n_cores"]
    rows2 = ncr * npad  # padded layer-2 table rows

    nc = bacc.Bacc("TRN2", target_bir_lowering=False, debug=False,
                   num_devices=ncr)
    x_full = nc.dram_tensor("x_full", [n_nodes, D], f32, kind="ExternalInput")
    xT = nc.dram_tensor("xT", [P, npad], f32, kind="ExternalInput")
    idx_d = [nc.dram_tensor(f"idx{l}", [P, tts[l] * 8], i16,
                            kind="ExternalInput") for l in range(2)]
    dl_d = [nc.dram_tensor(f"dl{l}", [P, tts[l]], f32,
                           kind="ExternalInput") for l in range(2)]
    wts = nc.dram_tensor("wts", [P, 4 * D], f32, kind="ExternalInput")
    wcat = nc.dram_tensor("wcat", [P, N_HEAD], f32, kind="ExternalInput")
    bcat = nc.dram_tensor("bcat", [1, N_HEAD], f32, kind="ExternalInput")
    vecs = nc.dram_tensor("vecs", [P, 8], f32, kind="ExternalInput")
    out_all = nc.dram_tensor("out_all", [npad, N_HEAD], f32,
                             kind="ExternalOutput")

    with tile.TileContext(nc) as tc:
        with tc.tile_pool(name="const", bufs=1) as cp, \
             tc.tile_pool(name="work", bufs=2) as wp, \
             tc.tile_pool(name="psum", bufs=2, space="PSUM") as pp, \
             tc.tile_pool(name="dram", bufs=1, space="DRAM") as dp:

            ident = cp.tile([P, P], f32)
            make_identity(nc, ident[:])
            iota_i = cp.tile([P, P], mybir.dt.int32)
            nc.gpsimd.iota(iota_i[:], pattern=[[1, P]], base=0,
                           channel_multiplier=0)
            iota = cp.tile([P, P], f32)
            nc.vector.tensor_copy(iota[:], iota_i[:])
            ones_r = cp.tile([1, P], f32)
            nc.vector.memset(ones_r[:], 1.0)

            w_sb = cp.tile([P, 4 * D], f32)
            nc.sync.dma_start(out=w_sb[:], in_=wts[:])
            wcat_sb = cp.tile([P, N_HEAD], f32)
            nc.sync.dma_start(out=wcat_sb[:], in_=wcat[:])
            bcat_sb = cp.tile([1, N_HEAD], f32)
            nc.sync.dma_start(out=bcat_sb[:], in_=bcat[:])
            v_sb = cp.tile([P, 8], f32)
            nc.sync.dma_start(out=v_sb[:], in_=vecs[:])
            # vecs columns: s1,t1,so,t2,s3,t3,b4
            s_mlp = [v_sb[:, 0:1], v_sb[:, 4:5]]
            t_mlp = [v_sb[:, 1:2], v_sb[:, 5:6]]
            s_out = [v_sb[:, 2:3], None]
            t_out = [v_sb[:, 3:4], v_sb[:, 6:7]]

            bT = cp.tile([P, npad], f32)  # layer-1 output, channel-major
            own_b = dp.tile([npad, D], f32)
            b_full = dp.tile([rows2, D], f32, addr_space="Shared")

            for l in range(2):
                tt = tts[l]
                w1 = w_sb[:, (2 * l) * D:(2 * l + 1) * D]
                w2 = w_sb[:, (2 * l + 1) * D:(2 * l + 2) * D]
                idx_sb = wp.tile([P, tt * 8], i16, tag="idx", bufs=1)
                nc.sync.dma_start(out=idx_sb[:], in_=idx_d[l][:])
                dl_sb = wp.tile([P, tt], f32, tag="dl", bufs=1)
                nc.sync.dma_start(out=dl_sb[:], in_=dl_d[l][:])
                if l == 0 or "nocoll" in _DBG:
                    tab_lo = x_full[0:min(split, n_nodes), :]
                    tab_hi = (x_full[split:n_nodes, :]
                              if n_nodes > split else None)
                else:
                    tab_lo = b_full[0:min(split, rows2), :]
                    tab_hi = (b_full[split:rows2, :]
                              if rows2 > split else None)

                for sc in plans[l]:
                    t0, nA, nB = sc["t0"], sc["nA"], sc["nB"]
                    nt, nblk, blk0 = nA + nB, sc["nblk"], sc["blk0"]
                    g_sb = wp.tile([P, max(nt, 1) * P], f32, tag="G", bufs=2)
                    s_sb = wp.tile([P, max(nt, 1) * P], f32, tag="S", bufs=2)
                    if "smallgather" in _DBG:
                        for kb, a0, ta_, b0, tb_ in sc["blocks"]:
                            _emit_gather(nc, g_sb, a0, ta_, tab_lo, idx_sb,
                                         t0 + a0)
                            _emit_gather(nc, g_sb, b0, tb_, tab_hi, idx_sb,
                                         t0 + b0)
                    else:
                        _emit_gather(nc, g_sb, 0, nA, tab_lo, idx_sb, t0)
                        _emit_gather(nc, g_sb, nA, nB, tab_hi, idx_sb, t0 + nA)
                    if nt and "noiseq" not in _DBG:
                        nc.vector.tensor_tensor(
                            out=s_sb[:, :nt * P].rearrange(
                                "p (t e) -> p t e", e=P),
                            in0=iota[:, None, :].to_broadcast([P, nt, P]),
                            in1=dl_sb[:, t0:t0 + nt, None].to_broadcast(
                                [P, nt, P]),
                            op=mybir.AluOpType.is_equal,
                        )

                    ncols = nblk * P
                    if l == 0:
                        seed = wp.tile([P, ncols], f32, tag="xc", bufs=2)
                        nc.sync.dma_start(
                            out=seed[:], in_=xT[:, blk0 * P:blk0 * P + ncols])
                    else:
                        seed = bT[:, blk0 * P:blk0 * P + ncols]
                    h_sb = wp.tile([P, ncols], f32, tag="h", bufs=2)
                    for kb, a0, ta_, b0, tb_ in sc["blocks"]:
                        ps = pp.tile([P, P], f32, tag="agg", bufs=4,
                                     padded_shape=[P, 512])
                        ntile_b = ta_ + tb_
                        tlist = (list(range(a0, a0 + ta_))
                                 + list(range(b0, b0 + tb_)))
                        if "nogather" in _DBG or "nomm" in _DBG:
                            tlist, ntile_b = [], 0
                        nc.tensor.matmul(
                            ps[:], lhsT=ident[:],
                            rhs=seed[:, kb * P:(kb + 1) * P],
                            start=True, stop=(ntile_b == 0))
                        for j, lt in enumerate(tlist):
                            nc.tensor.matmul(
                                ps[:],
                                lhsT=g_sb[:, lt * P:(lt + 1) * P],
                                rhs=s_sb[:, lt * P:(lt + 1) * P],
                                start=False, stop=(j == ntile_b - 1))
                        nc.scalar.copy(h_sb[:, kb * P:(kb + 1) * P], ps[:])

                    z1 = pp.tile([P, ncols], f32, tag="z1", bufs=2)
                    nc.tensor.matmul(z1[:], lhsT=w1, rhs=h_sb[:],
                                     start=True, stop=True)
                    a1 = wp.tile([P, ncols], f32, tag="a1", bufs=2)
                    nc.scalar.activation(
                        a1[:], z1[:], mybir.ActivationFunctionType.Relu,
                        bias=t_mlp[l], scale=s_mlp[l])
                    z2 = pp.tile([P, ncols], f32, tag="z2", bufs=2)
                    nc.tensor.matmul(z2[:], lhsT=w2, rhs=a1[:],
                                     start=True, stop=True)

                    if l == 0:
                        nc.scalar.activation(
                            bT[:, blk0 * P:blk0 * P + ncols], z2[:],
                            mybir.ActivationFunctionType.Relu,
                            bias=t_out[0], scale=s_out[0])
                        stg = wp.tile([P, ncols], f32, tag="stg", bufs=2)
                        if "notrans" in _DBG:
                            nc.vector.tensor_copy(
                                stg[:], bT[:, blk0 * P:blk0 * P + ncols])
                        else:
                            for kb in range(nblk):
                                tp = pp.tile([P, P], f32, tag="agg", bufs=4,
                                             padded_shape=[P, 512])
                                nc.tensor.transpose(
                                    tp[:],
                                    bT[:, (blk0 + kb) * P:(blk0 + kb + 1) * P],
                                    ident[:])
                                nc.vector.tensor_copy(
                                    stg[:, kb * P:(kb + 1) * P], tp[:])
                        nc.sync.dma_start(
                            out=own_b[:].rearrange(
                                "(t p) e -> p t e", p=P)[:, blk0:blk0 + nblk],
                            in_=stg[:].rearrange("p (t e) -> p t e", e=D))
                    else:
                        hf = wp.tile([P, ncols], f32, tag="hf", bufs=2)
                        nc.scalar.activation(
                            hf[:], z2[:],
                            mybir.ActivationFunctionType.Identity,
                            bias=t_out[1], scale=1.0)
                        osb = wp.tile([P, nblk * N_HEAD], f32, tag="osb",
                                      bufs=2)
                        if "noheads" in _DBG:
                            nc.vector.memset(osb[:], 0.0)
                        for kb in range(nblk if "noheads" not in _DBG else 0):
                            ph = pp.tile([P, N_HEAD], f32, tag="z1", bufs=2)
                            nc.tensor.matmul(
                                ph[:], lhsT=hf[:, kb * P:(kb + 1) * P],
                                rhs=wcat_sb[:], start=True, stop=False)
                            nc.tensor.matmul(
                                ph[:], lhsT=ones_r[:], rhs=bcat_sb[:],
                                start=False, stop=True)
                            o0 = kb * N_HEAD
                            # log_softmax over cols 0:40
                            m1 = wp.tile([P, 1], f32, tag="m1", bufs=3)
                            nc.vector.reduce_max(
                                m1[:], ph[:, 0:40],
                                axis=mybir.AxisListType.X, negate=True)
                            e1 = wp.tile([P, 40], f32, tag="e1", bufs=3)
                            nc.scalar.activation(
                                e1[:], ph[:, 0:40],
                                mybir.ActivationFunctionType.Exp,
                                bias=m1[:])
                            s1_ = wp.tile([P, 1], f32, tag="s1", bufs=3)
                            nc.vector.reduce_sum(
                                s1_[:], e1[:], axis=mybir.AxisListType.X)
                            ls = wp.tile([P, 1], f32, tag="ls", bufs=3)
                            nc.scalar.activation(
                                ls[:], s1_[:],
                                mybir.ActivationFunctionType.Ln)
                            lm = wp.tile([P, 1], f32, tag="lm", bufs=3)
                            nc.vector.tensor_sub(lm[:], m1[:], ls[:])
                            nc.vector.tensor_scalar(
                                out=osb[:, o0:o0 + 40], in0=ph[:, 0:40],
                                scalar1=lm[:], scalar2=None,
                                op0=mybir.AluOpType.add)
                            # softmax over cols 40:80
                            m2 = wp.tile([P, 1], f32, tag="m2", bufs=3)
                            nc.vector.reduce_max(
                                m2[:], ph[:, 40:80],
                                axis=mybir.AxisListType.X, negate=True)
                            e2 = wp.tile([P, 40], f32, tag="e2", bufs=3)
                            nc.scalar.activation(
                                e2[:], ph[:, 40:80],
                                mybir.ActivationFunctionType.Exp,
                                bias=m2[:])
                            s2_ = wp.tile([P, 1], f32, tag="s2", bufs=3)
                            nc.vector.reduce_sum(
                                s2_[:], e2[:], axis=mybir.AxisListType.X)
                            r2 = wp.tile([P, 1], f32, tag="r2", bufs=3)
                            nc.vector.reciprocal(r2[:], s2_[:])
                            nc.vector.tensor_scalar(
                                out=osb[:, o0 + 40:o0 + 80], in0=e2[:],
                                scalar1=r2[:], scalar2=None,
                                op0=mybir.AluOpType.mult)
                            # sigmoid heads
                            nc.scalar.activation(
                                osb[:, o0 + 80:o0 + 82], ph[:, 80:82],
                                mybir.ActivationFunctionType.Sigmoid)
                        nc.sync.dma_start(
                            out=out_all[:].rearrange(
                                "(t p) e -> p t e", p=P)[:, blk0:blk0 + nblk],
                            in_=osb[:].rearrange("p (t e) -> p t e",
                                                 e=N_HEAD))

                if l == 0 and "nocoll" not in _DBG:
                    nc.gpsimd.collective_compute(
                        "AllGather", mybir.AluOpType.bypass,
                        replica_groups=[list(range(ncr))],
                        ins=[own_b[:].opt()], outs=[b_full[:].opt()],
                    )
    nc.compile()
    return nc


# --------------------------------------------------------------------------
# host orchestration
# --------------------------------------------------------------------------

def _prepare(x, edge_index, weights, n_cores=N_CORES, split=SPLIT):
    n_nodes = x.shape[0]
    assert n_nodes % n_cores == 0
    npc = n_nodes // n_cores
    nb = -(-npc // P)
    npad = nb * P
    cfg = {"n_nodes": n_nodes, "n_cores": n_cores, "npc": npc, "nb": nb,
           "npad": npad, "split": split}

    src = np.asarray(edge_index[0], np.int64)
    dst = np.asarray(edge_index[1], np.int64)
    core_of = dst // npc
    dst_local = dst - core_of * npc

    plan1, idx1, dl1, tt1 = _plan_layer(src, dst_local, core_of, cfg)
    # layer-2 table rows include npad-npc pad rows per core
    rows2 = (src // npc) * npad + (src % npc)
    plan2, idx2, dl2, tt2 = _plan_layer(rows2, dst_local, core_of, cfg)
    cfg["tt1"], cfg["tt2"] = tt1, tt2
    return cfg, (plan1, plan2), (idx1, dl1, idx2, dl2)


def _fold_weights(w):
    s = np.float32(1.0 / math.sqrt(1.0 + BN_EPS))
    s1 = w["c1_g1"] * s
    t1 = w["c1_b1"] * s1 + w["c1_be1"]
    so = w["bn_g"] * s
    t2 = w["c1_b2"] * so + w["bn_b"]
    s3 = w["c2_g1"] * s
    t3 = w["c2_b1"] * s3 + w["c2_be1"]
    b4 = w["c2_b2"]
    vecs = np.stack([s1, t1, so, t2, s3, t3, b4,
                     np.zeros_like(s1)], axis=1).astype(np.float32)
    wts = np.concatenate([w["c1_W1"], w["c1_W2"], w["c2_W1"], w["c2_W2"]],
                         axis=1).astype(np.float32)
    wcat = np.concatenate([w["cls_W"], w["sim_W"], w["hom_W"], w["ent_W"]],
                          axis=1).astype(np.float32)
    bcat = np.concatenate([w["cls_b"], w["sim_b"], w["hom_b"], w["ent_b"]]
                          ).astype(np.float32)[None, :]
    return wts, wcat, bcat, vecs


def _run(x, edge_index, weights, n_cores=N_CORES, split=SPLIT):
    x = np.ascontiguousarray(np.asarray(x, np.float32))
    key = (x.shape, hash(np.asarray(edge_index).tobytes()), n_cores, split)
    if key not in _CACHE:
        cfg, plans, arrs = _prepare(x, edge_index, weights, n_cores, split)
        nc = _build_program(cfg, plans)
        _CACHE.clear()
        _CACHE[key] = (cfg, nc, arrs)
    cfg, nc, (idx1, dl1, idx2, dl2) = _CACHE[key]

    npc, npad = cfg["npc"], cfg["npad"]
    wts, wcat, bcat, vecs = _fold_weights(weights)
    in_maps = []
    for c in range(n_cores):
        xt = np.zeros((P, npad), np.float32)
        xt[:, :npc] = x[c * npc:(c + 1) * npc].T
        in_maps.append({
            "x_full": x, "xT": xt,
            "idx0": idx1[c], "dl0": dl1[c],
            "idx1": idx2[c], "dl1": dl2[c],
            "wts": wts, "wcat": wcat, "bcat": bcat, "vecs": vecs,
        })
    res = bass_utils.run_bass_kernel_spmd(
        nc, in_maps, core_ids=list(range(n_cores)))
    full = np.concatenate(
        [res.results[c]["out_all"][:npc] for c in range(n_cores)], axis=0)
    return (np.ascontiguousarray(full[:, 0:40]),
            np.ascontiguousarray(full[:, 40:80]),
            np.ascontiguousarray(full[:, 80]),
            np.ascontiguousarray(full[:, 81]))


def bench_ns(inputs, iters=10):
    """Wall-clock repeated executions of the cached compiled NEFF on the 8
    cores (inputs device-resident, async dispatch pipelined). Returns the
    mean per-iteration time in ns — an upper bound on HW exec time that
    includes dispatch overhead."""
    import time

    import jax
    import numpy as _np
    from jax.experimental.shard_map import shard_map
    from jax.sharding import Mesh, PartitionSpec

    from concourse import bass2jax
    from concourse.bass2jax import _bass_exec_p, partition_id_tensor

    x = np.ascontiguousarray(np.asarray(inputs["x"], np.float32))
    edge_index = np.asarray(inputs["edge_index"], np.int64)
    weights = {k: np.asarray(v, np.float32) for k, v in inputs.items()
               if k not in ("x", "edge_index")}
    # populate cache + in_maps exactly as _run does
    key = (x.shape, hash(np.asarray(edge_index).tobytes()), N_CORES, SPLIT)
    if key not in _CACHE:
        _run(x, edge_index, weights)
    cfg, nc, (idx1, dl1, idx2, dl2) = _CACHE[key]
    npc, npad = cfg["npc"], cfg["npad"]
    wts, wcat, bcat, vecs = _fold_weights(weights)
    in_maps = []
    for c in range(N_CORES):
        xt = np.zeros((P, npad), np.float32)
        xt[:, :npc] = x[c * npc:(c + 1) * npc].T
        in_maps.append({
            "x_full": x, "xT": xt,
            "idx0": idx1[c], "dl0": dl1[c],
            "idx1": idx2[c], "dl1": dl2[c],
            "wts": wts, "wcat": wcat, "bcat": bcat, "vecs": vecs,
        })

    bass2jax.install_neuronx_cc_hook()
    in_names, out_names, out_avals, zero_outs = [], [], [], []
    partition_name = (nc.partition_id_tensor.name
                      if nc.partition_id_tensor else None)
    for alloc in nc.m.functions[0].allocations:
        if not isinstance(alloc, mybir.MemoryLocationSet):
            continue
        name = alloc.memorylocations[0].name
        if alloc.kind == "ExternalInput":
            if name != partition_name:
                in_names.append(name)
        elif alloc.kind == "ExternalOutput":
            shape = tuple(alloc.tensor_shape)
            dtype = mybir.dt.np(alloc.dtype)
            out_names.append(name)
            out_avals.append(jax.core.ShapedArray(shape, dtype))
            zero_outs.append(_np.zeros(shape, dtype))
    n_params = len(in_names)
    all_in_names = list(in_names) + list(out_names)
    if partition_name is not None:
        all_in_names.append(partition_name)

    def _body(*args):
        operands = list(args)
        if partition_name is not None:
            operands.append(partition_id_tensor())
        outs = _bass_exec_p.bind(
            *operands,
            out_avals=tuple(out_avals),
            in_names=tuple(all_in_names),
            out_names=tuple(out_names),
            lowering_input_output_aliases=(),
            sim_require_finite=True,
            sim_require_nnan=True,
            nc=nc,
        )
        return tuple(outs)

    devices = jax.devices()[:N_CORES]
    mesh = Mesh(_np.asarray(devices), ("core",))
    n_outs = len(out_avals)
    sharded = jax.jit(
        shard_map(_body, mesh=mesh,
                  in_specs=(PartitionSpec("core"),) * (n_params + n_outs),
                  out_specs=(PartitionSpec("core"),) * n_outs,
                  check_rep=False),
        keep_unused=True,
    )
    concat_in = [
        _np.concatenate([_np.asarray(in_maps[c][nm])
                         for c in range(N_CORES)], axis=0)
        for nm in in_names
    ]
    concat_zeros = [
        _np.zeros((N_CORES * z.shape[0], *z.shape[1:]), z.dtype)
        for z in zero_outs
    ]
    args = [jax.device_put(a) for a in concat_in + concat_zeros]
    # warmup (compiles)
    out = sharded(*args)
    jax.block_until_ready(out)
    t0 = time.perf_counter()
    outs = [sharded(*args) for _ in range(iters)]
    jax.block_until_ready(outs)
    dt = (time.perf_counter() - t0) / iters
    return dt * 1e9


def kernel(x, edge_index,
           c1_W1, c1_b1, c1_g1, c1_be1, c1_W2, c1_b2,
           c2_W1, c2_b1, c2_g1, c2_be1, c2_W2, c2_b2,
           bn_g, bn_b,
           cls_W, cls_b, sim_W, sim_b, hom_W, hom_b, ent_W, ent_b):
    weights = dict(
        c1_W1=np.asarray(c1_W1, np.float32), c1_b1=np.asarray(c1_b1, np.float32),
        c1_g1=np.asarray(c1_g1, np.float32), c1_be1=np.asarray(c1_be1, np.float32),
        c1_W2=np.asarray(c1_W2, np.float32), c1_b2=np.asarray(c1_b2, np.float32),
        c2_W1=np.asarray(c2_W1, np.float32), c2_b1=np.asarray(c2_b1, np.float32),
        c2_g1=np.asarray(c2_g1, np.float32), c2_be1=np.asarray(c2_be1, np.float32),
        c2_W2=np.asarray(c2_W2, np.float32), c2_b2=np.asarray(c2_b2, np.float32),
        bn_g=np.asarray(bn_g, np.float32), bn_b=np.asarray(bn_b, np.float32),
        cls_W=np.asarray(cls_W, np.float32), cls_b=np.asarray(cls_b, np.float32),
        sim_W=np.asarray(sim_W, np.float32), sim_b=np.asarray(sim_b, np.float32),
        hom_W=np.asarray(hom_W, np.float32), hom_b=np.asarray(hom_b, np.float32),
        ent_W=np.asarray(ent_W, np.float32), ent_b=np.asarray(ent_b, np.float32),
    )
    return _run(np.asarray(x, np.float32), np.asarray(edge_index, np.int64),
                weights)


# revision 17
# speedup vs baseline: 1.3813x; 1.3813x over previous
"""Trainium2 Bass kernel for nn_AuxiliaryGIN (2-layer GIN + 4 output heads).

Sharding (8 NeuronCores): nodes row-partitioned 6250/core (edges partitioned
by destination node so scatter-adds stay local); 128x128 weights replicated;
halo = full bf16 feature table in HBM per layer (layer 2's is assembled with
an AllGather).

Per core, per layer:
  * edges are grouped by destination block (128 nodes) and split into
    lo/hi source tables (dma_gather indices are int16, so a feature table is
    limited to 32768 rows)
  * batched multi-packet dma_gather pulls bf16 feature rows into SBUF tiles
    G [128 edges, 128 d]
  * a bf16 one-hot S^T [128 edges, 128 dst] per tile is built on the vector
    engine with a broadcast is_equal against an iota row
  * agg^T[d, dst] = sum_tiles G^T @ S^T accumulates in fp32 PSUM (one PSUM
    bank holds a whole superchunk of 4 destination blocks); GIN's "+x" term
    is a fused f32 add when copying PSUM out
  * the GIN MLP runs channel-major: W is the natural lhsT, BN/bias/ReLU are
    per-partition scalar-engine activations
  * layer 1 additionally computes its output node-major (lhsT = a1 block)
    so the bf16 row-major halo table needs no transposes; an AllGather
    assembles the full table for layer 2's gathers
  * heads: per node block one matmul against the concatenated head weights
    into a shared PSUM slab, then log_softmax / softmax / sigmoid batched
    across the superchunk
"""

import math
import os

import ml_dtypes
import numpy as np

import concourse.bacc as bacc
import concourse.bass as bass
import concourse.mybir as mybir
import concourse.tile as tile
from concourse import bass_utils

_DBG = set(os.environ.get("KDBG", "").split(",")) - {""}

P = 128
N_CORES = 8
D = 128
N_HEAD = 82  # 40 cls + 40 sim + 1 hom + 1 ent
SPLIT = 32768  # dma_gather int16 index limit per table
NBLK_SC = 4  # node blocks per superchunk (gather/MLP granularity)
GMAX = 48  # tiles per dma_gather (multi-packet; 64 verified safe, 128 not)
BN_EPS = 1e-5

f32 = mybir.dt.float32
bf16 = mybir.dt.bfloat16
i16 = mybir.dt.int16
npbf = ml_dtypes.bfloat16

_CACHE: dict = {}


# --------------------------------------------------------------------------
# host-side planning
# --------------------------------------------------------------------------

def _plan_layer(src_rows, dst_local_all, core_of_dst, cfg):
    """Group each core's edges by destination block, split lo/hi by source
    row, and pad per-(block, table) tile counts to the max over cores so the
    SPMD program is shape-uniform.

    Returns (plan, idx16 [n_cores, 128, TT*8], dstloc [n_cores, 128, TT]).
    plan is a list of superchunks:
      {"t0": int, "nA": int, "nB": int, "blk0": int, "nblk": int,
       "blocks": [(kb, a0, ta, b0, tb), ...]}  # tile offsets local to sc
    """
    nc_, nb, split = cfg["n_cores"], cfg["nb"], cfg["split"]

    lo_rows = [[None] * nb for _ in range(nc_)]
    lo_dloc = [[None] * nb for _ in range(nc_)]
    hi_rows = [[None] * nb for _ in range(nc_)]
    hi_dloc = [[None] * nb for _ in range(nc_)]
    blk_all = dst_local_all >> 7
    dloc_all = dst_local_all & 127
    is_hi = src_rows >= split
    for c in range(nc_):
        cm = core_of_dst == c
        for b in range(nb):
            m = cm & (blk_all == b)
            ml = m & ~is_hi
            mh = m & is_hi
            lo_rows[c][b] = src_rows[ml]
            lo_dloc[c][b] = dloc_all[ml]
            hi_rows[c][b] = src_rows[mh] - split
            hi_dloc[c][b] = dloc_all[mh]

    ta = [max(-(-len(lo_rows[c][b]) // P) for c in range(nc_))
          for b in range(nb)]
    tb = [max(-(-len(hi_rows[c][b]) // P) for c in range(nc_))
          for b in range(nb)]

    plan = []
    t0 = 0
    for s0 in range(0, nb, NBLK_SC):
        blocks = list(range(s0, min(s0 + NBLK_SC, nb)))
        nA = sum(ta[b] for b in blocks)
        nB = sum(tb[b] for b in blocks)
        binfo = []
        a_off, b_off = 0, nA
        for kb, b in enumerate(blocks):
            binfo.append((kb, a_off, ta[b], b_off, tb[b]))
            a_off += ta[b]
            b_off += tb[b]
        plan.append({"t0": t0, "nA": nA, "nB": nB, "blocks": binfo,
                     "blk0": s0, "nblk": len(blocks)})
        t0 += nA + nB
    tt = t0

    idx16 = np.zeros((nc_, P, tt * 8), np.int16)
    dstloc = np.full((nc_, P, tt), -1.0, npbf)

    def fill(c, tile0, ntiles, rows, dloc):
        if ntiles == 0:
            return
        n_pad = ntiles * P
        r = np.zeros(n_pad, np.int64)
        r[:len(rows)] = rows
        d_ = np.full(n_pad, -1.0, npbf)
        d_[:len(dloc)] = dloc
        idx16[c, :16, tile0 * 8:(tile0 + ntiles) * 8] = \
            r.astype(np.int16).reshape(-1, 16).T
        dstloc[c, :, tile0:tile0 + ntiles] = d_.reshape(-1, P).T

    for c in range(nc_):
        for sc in plan:
            for kb, a0, ta_, b0, tb_ in sc["blocks"]:
                b = sc["blk0"] + kb
                fill(c, sc["t0"] + a0, ta_, lo_rows[c][b], lo_dloc[c][b])
                fill(c, sc["t0"] + b0, tb_, hi_rows[c][b], hi_dloc[c][b])
    # q7 cores read idx partitions in groups of 16, replicated 8x
    idx16[:, 16:, :] = np.tile(idx16[:, :16, :], (1, 7, 1))
    return plan, idx16, dstloc, tt


# --------------------------------------------------------------------------
# device program
# --------------------------------------------------------------------------

def _emit_gather(nc, g_sb, col0, ntiles, table, idx_sb, tile0):
    if ntiles == 0 or "nogather" in _DBG:
        return
    for c0 in range(0, ntiles, GMAX):
        nt = min(GMAX, ntiles - c0)
        n = nt * P
        nc.gpsimd.dma_gather(
            out_ap=g_sb[:, (col0 + c0) * P:(col0 + c0 + nt) * P].rearrange(
                "p (t e) -> p t e", e=D),
            in_ap=table,
            idxs_ap=idx_sb[:, (tile0 + c0) * 8:(tile0 + c0 + nt) * 8],
            num_idxs=n,
            num_idxs_reg=n,
            elem_size=D,
            single_packet=False,
        )


def _build_program(cfg, plans):
    n_nodes, npad = cfg["n_nodes"], cfg["npad"]
    nb, split = cfg["nb"], cfg["split"]
    tts = [cfg["tt1"], cfg["tt2"]]
    ncr = cfg["n_cores"]
    rows2 = ncr * npad  # padded layer-2 table rows

    nc = bacc.Bacc("TRN2", target_bir_lowering=False, debug=False,
                   num_devices=ncr)
    x_bf = nc.dram_tensor("x_bf", [n_nodes, D], bf16, kind="ExternalInput")
    xT = nc.dram_tensor("xT", [P, npad], f32, kind="ExternalInput")
    idx_d = [nc.dram_tensor(f"idx{l}", [P, tts[l] * 8], i16,
                            kind="ExternalInput") for l in range(2)]
    dl_d = [nc.dram_tensor(f"dl{l}", [P, tts[l]], bf16,
                           kind="ExternalInput") for l in range(2)]
    wts = nc.dram_tensor("wts", [P, 4 * D], f32, kind="ExternalInput")
    wcat = nc.dram_tensor("wcat", [P, N_HEAD], f32, kind="ExternalInput")
    bcat = nc.dram_tensor("bcat", [1, N_HEAD], f32, kind="ExternalInput")
    vecs = nc.dram_tensor("vecs", [P, 8], f32, kind="ExternalInput")
    trow = nc.dram_tensor("trow", [1, D], f32, kind="ExternalInput")
    out_all = nc.dram_tensor("out_all", [npad, N_HEAD], f32,
                             kind="ExternalOutput")

    with tile.TileContext(nc) as tc:
        with tc.tile_pool(name="const", bufs=1) as cp, \
             tc.tile_pool(name="work", bufs=2) as wp, \
             tc.tile_pool(name="psum", bufs=2, space="PSUM") as pp, \
             tc.tile_pool(name="dram", bufs=1, space="DRAM") as dp:

            iota_i = cp.tile([P, P], mybir.dt.int32)
            nc.gpsimd.iota(iota_i[:], pattern=[[1, P]], base=0,
                           channel_multiplier=0)
            iota_bf = cp.tile([P, P], bf16)
            nc.vector.tensor_copy(iota_bf[:], iota_i[:])
            ones_r = cp.tile([1, P], f32)
            nc.vector.memset(ones_r[:], 1.0)

            w_sb = cp.tile([P, 4 * D], f32)
            nc.sync.dma_start(out=w_sb[:], in_=wts[:])
            wcat_sb = cp.tile([P, N_HEAD], f32)
            nc.sync.dma_start(out=wcat_sb[:], in_=wcat[:])
            bcat_sb = cp.tile([1, N_HEAD], f32)
            nc.sync.dma_start(out=bcat_sb[:], in_=bcat[:])
            v_sb = cp.tile([P, 8], f32)
            nc.sync.dma_start(out=v_sb[:], in_=vecs[:])
            trow_sb = cp.tile([1, D], f32)
            nc.sync.dma_start(out=trow_sb[:], in_=trow[:])
            # vecs columns: s1,t1,t2,_,s3,t3,b4  (W2_l1 has so pre-folded)
            s_mlp = [v_sb[:, 0:1], v_sb[:, 4:5]]
            t_mlp = [v_sb[:, 1:2], v_sb[:, 5:6]]
            t_out = [v_sb[:, 2:3], v_sb[:, 6:7]]

            bT = cp.tile([P, npad], f32)  # layer-1 output, channel-major
            own_b = dp.tile([npad, D], bf16)
            b_full = dp.tile([rows2, D], bf16, addr_space="Shared")

            for l in range(2):
                tt = tts[l]
                w1 = w_sb[:, (2 * l) * D:(2 * l + 1) * D]
                w2 = w_sb[:, (2 * l + 1) * D:(2 * l + 2) * D]
                idx_sb = wp.tile([P, tt * 8], i16, tag="idx", bufs=1)
                nc.sync.dma_start(out=idx_sb[:], in_=idx_d[l][:])
                dl_sb = wp.tile([P, tt], bf16, tag="dl", bufs=1)
                nc.sync.dma_start(out=dl_sb[:], in_=dl_d[l][:])
                if l == 0 or "nocoll" in _DBG:
                    tab_lo = x_bf[0:min(split, n_nodes), :]
                    tab_hi = (x_bf[split:n_nodes, :]
                              if n_nodes > split else None)
                else:
                    tab_lo = b_full[0:min(split, rows2), :]
                    tab_hi = (b_full[split:rows2, :]
                              if rows2 > split else None)

                for sc in plans[l]:
                    t0, nA, nB = sc["t0"], sc["nA"], sc["nB"]
                    nt, nblk, blk0 = nA + nB, sc["nblk"], sc["blk0"]
                    ncols = nblk * P
                    g_sb = wp.tile([P, max(nt, 1) * P], bf16, tag="G", bufs=2)
                    s_sb = wp.tile([P, max(nt, 1) * P], bf16, tag="S", bufs=2)
                    _emit_gather(nc, g_sb, 0, nA, tab_lo, idx_sb, t0)
                    _emit_gather(nc, g_sb, nA, nB, tab_hi, idx_sb, t0 + nA)
                    if nt and "noiseq" not in _DBG:
                        nc.vector.tensor_tensor(
                            out=s_sb[:, :nt * P].rearrange(
                                "p (t e) -> p t e", e=P),
                            in0=iota_bf[:, None, :].to_broadcast([P, nt, P]),
                            in1=dl_sb[:, t0:t0 + nt, None].to_broadcast(
                                [P, nt, P]),
                            op=mybir.AluOpType.is_equal,
                        )

                    if l == 0:
                        seed = wp.tile([P, ncols], f32, tag="xc", bufs=2)
                        nc.sync.dma_start(
                            out=seed[:], in_=xT[:, blk0 * P:blk0 * P + ncols])
                    else:
                        seed = bT[:, blk0 * P:blk0 * P + ncols]

                    # one PSUM bank accumulates the whole superchunk's agg
                    ps = pp.tile([P, ncols], f32, tag="agg", bufs=2)
                    fused = all(ta_ + tb_ > 0
                                for _, _, ta_, _, tb_ in sc["blocks"])
                    for kb, a0, ta_, b0, tb_ in sc["blocks"]:
                        tlist = (list(range(a0, a0 + ta_))
                                 + list(range(b0, b0 + tb_)))
                        if "nogather" in _DBG or "nomm" in _DBG:
                            tlist = []
                        if not tlist:
                            continue
                        for j, lt in enumerate(tlist):
                            nc.tensor.matmul(
                                ps[:, kb * P:(kb + 1) * P],
                                lhsT=g_sb[:, lt * P:(lt + 1) * P],
                                rhs=s_sb[:, lt * P:(lt + 1) * P],
                                start=(j == 0), stop=(j == len(tlist) - 1))
                    h_sb = wp.tile([P, ncols], f32, tag="h", bufs=2)
                    if fused and "nogather" not in _DBG \
                            and "nomm" not in _DBG:
                        nc.vector.tensor_add(h_sb[:], ps[:], seed[:])
                    else:
                        for kb, a0, ta_, b0, tb_ in sc["blocks"]:
                            sl = slice(kb * P, (kb + 1) * P)
                            if (ta_ + tb_ > 0 and "nogather" not in _DBG
                                    and "nomm" not in _DBG):
                                nc.vector.tensor_add(
                                    h_sb[:, sl], ps[:, sl], seed[:, sl])
                            else:
                                nc.vector.tensor_copy(
                                    h_sb[:, sl], seed[:, sl])

                    z1 = pp.tile([P, ncols], f32, tag="z1", bufs=2)
                    nc.tensor.matmul(z1[:], lhsT=w1, rhs=h_sb[:],
                                     start=True, stop=True)
                    a1 = wp.tile([P, ncols], f32, tag="a1", bufs=2)
                    nc.scalar.activation(
                        a1[:], z1[:], mybir.ActivationFunctionType.Relu,
                        bias=t_mlp[l], scale=s_mlp[l])
                    z2 = pp.tile([P, ncols], f32, tag="z2", bufs=2)
                    nc.tensor.matmul(z2[:], lhsT=w2, rhs=a1[:],
                                     start=True, stop=True)

                    if l == 0:
                        # channel-major for layer 2's seed + MLP
                        nc.scalar.activation(
                            bT[:, blk0 * P:blk0 * P + ncols], z2[:],
                            mybir.ActivationFunctionType.Relu,
                            bias=t_out[0], scale=1.0)
                        # node-major bf16 rows for the halo table
                        stg = wp.tile([P, ncols], bf16, tag="stg", bufs=2)
                        for kb in range(nblk):
                            psr = pp.tile([P, P], f32, tag="ph", bufs=2)
                            nc.tensor.matmul(
                                psr[:], lhsT=a1[:, kb * P:(kb + 1) * P],
                                rhs=w2, start=True, stop=False)
                            nc.tensor.matmul(
                                psr[:], lhsT=ones_r[:], rhs=trow_sb[:],
                                start=False, stop=True)
                            nc.scalar.activation(
                                stg[:, kb * P:(kb + 1) * P], psr[:],
                                mybir.ActivationFunctionType.Relu)
                        nc.sync.dma_start(
                            out=own_b[:].rearrange(
                                "(t p) e -> p t e", p=P)[:, blk0:blk0 + nblk],
                            in_=stg[:].rearrange("p (t e) -> p t e", e=D))
                    else:
                        hf = wp.tile([P, ncols], f32, tag="hf", bufs=2)
                        nc.scalar.activation(
                            hf[:], z2[:],
                            mybir.ActivationFunctionType.Identity,
                            bias=t_out[1], scale=1.0)
                        # heads: one PSUM slab per superchunk
                        ph = pp.tile([P, nblk * N_HEAD], f32, tag="ph",
                                     bufs=2)
                        for kb in range(nblk):
                            o0 = kb * N_HEAD
                            nc.tensor.matmul(
                                ph[:, o0:o0 + N_HEAD],
                                lhsT=hf[:, kb * P:(kb + 1) * P],
                                rhs=wcat_sb[:], start=True, stop=False)
                            nc.tensor.matmul(
                                ph[:, o0:o0 + N_HEAD],
                                lhsT=ones_r[:], rhs=bcat_sb[:],
                                start=False, stop=True)
                        zc = wp.tile([P, nblk * N_HEAD], f32, tag="zc",
                                     bufs=2)
                        nc.scalar.copy(zc[:], ph[:])
                        z3 = zc[:].rearrange("p (t e) -> p t e", e=N_HEAD)
                        osb = wp.tile([P, nblk * N_HEAD], f32, tag="osb",
                                      bufs=2)
                        o3 = osb[:].rearrange("p (t e) -> p t e", e=N_HEAD)
                        e1 = wp.tile([P, nblk * 40], f32, tag="e1", bufs=2)
                        e13 = e1[:].rearrange("p (t e) -> p t e", e=40)
                        m1 = wp.tile([P, nblk], f32, tag="m1", bufs=2)
                        s1_ = wp.tile([P, nblk], f32, tag="s1", bufs=2)
                        lm = wp.tile([P, nblk], f32, tag="lm", bufs=2)
                        # log_softmax over cols 0:40
                        nc.vector.reduce_max(
                            m1[:], z3[:, :, 0:40],
                            axis=mybir.AxisListType.X, negate=True)
                        nc.vector.tensor_tensor(
                            out=e13, in0=z3[:, :, 0:40],
                            in1=m1[:, :, None].to_broadcast([P, nblk, 40]),
                            op=mybir.AluOpType.add)
                        nc.scalar.activation(
                            e13, e13, mybir.ActivationFunctionType.Exp)
                        nc.vector.reduce_sum(
                            s1_[:], e13, axis=mybir.AxisListType.X)
                        nc.scalar.activation(
                            lm[:], s1_[:], mybir.ActivationFunctionType.Ln)
                        nc.vector.tensor_sub(lm[:], m1[:], lm[:])
                        nc.vector.tensor_tensor(
                            out=o3[:, :, 0:40], in0=z3[:, :, 0:40],
                            in1=lm[:, :, None].to_broadcast([P, nblk, 40]),
                            op=mybir.AluOpType.add)
                        # softmax over cols 40:80
                        nc.vector.reduce_max(
                            m1[:], z3[:, :, 40:80],
                            axis=mybir.AxisListType.X, negate=True)
                        nc.vector.tensor_tensor(
                            out=e13, in0=z3[:, :, 40:80],
                            in1=m1[:, :, None].to_broadcast([P, nblk, 40]),
                            op=mybir.AluOpType.add)
                        nc.scalar.activation(
                            e13, e13, mybir.ActivationFunctionType.Exp)
                        nc.vector.reduce_sum(
                            s1_[:], e13, axis=mybir.AxisListType.X)
                        nc.vector.reciprocal(lm[:], s1_[:])
                        nc.vector.tensor_tensor(
                            out=o3[:, :, 40:80], in0=e13,
                            in1=lm[:, :, None].to_broadcast([P, nblk, 40]),
                            op=mybir.AluOpType.mult)
                        # sigmoid heads
                        nc.scalar.activation(
                            o3[:, :, 80:82], z3[:, :, 80:82],
                            mybir.ActivationFunctionType.Sigmoid)
                        nc.sync.dma_start(
                            out=out_all[:].rearrange(
                                "(t p) e -> p t e", p=P)[:, blk0:blk0 + nblk],
                            in_=o3)

                if l == 0 and "nocoll" not in _DBG:
                    nc.gpsimd.collective_compute(
                        "AllGather", mybir.AluOpType.bypass,
                        replica_groups=[list(range(ncr))],
                        ins=[own_b[:].opt()], outs=[b_full[:].opt()],
                    )
    nc.compile()
    return nc


# --------------------------------------------------------------------------
# host orchestration
# --------------------------------------------------------------------------

def _prepare(x, edge_index, weights, n_cores=N_CORES, split=SPLIT):
    n_nodes = x.shape[0]
    assert n_nodes % n_cores == 0
    npc = n_nodes // n_cores
    nb = -(-npc // P)
    npad = nb * P
    cfg = {"n_nodes": n_nodes, "n_cores": n_cores, "npc": npc, "nb": nb,
           "npad": npad, "split": split}

    src = np.asarray(edge_index[0], np.int64)
    dst = np.asarray(edge_index[1], np.int64)
    core_of = dst // npc
    dst_local = dst - core_of * npc

    plan1, idx1, dl1, tt1 = _plan_layer(src, dst_local, core_of, cfg)
    # layer-2 table rows include npad-npc pad rows per core
    rows2 = (src // npc) * npad + (src % npc)
    plan2, idx2, dl2, tt2 = _plan_layer(rows2, dst_local, core_of, cfg)
    cfg["tt1"], cfg["tt2"] = tt1, tt2
    return cfg, (plan1, plan2), (idx1, dl1, idx2, dl2)


def _fold_weights(w):
    s = np.float32(1.0 / math.sqrt(1.0 + BN_EPS))
    s1 = w["c1_g1"] * s
    t1 = w["c1_b1"] * s1 + w["c1_be1"]
    so = w["bn_g"] * s
    t2 = w["c1_b2"] * so + w["bn_b"]
    s3 = w["c2_g1"] * s
    t3 = w["c2_b1"] * s3 + w["c2_be1"]
    b4 = w["c2_b2"]
    vecs = np.stack([s1, t1, t2, np.zeros_like(s1), s3, t3, b4,
                     np.zeros_like(s1)], axis=1).astype(np.float32)
    # fold the outer-BN scale into c1_W2 so both the channel-major and the
    # node-major layer-1 outputs are relu(a1 @ W2' + t2)
    w2p = (w["c1_W2"] * so[None, :]).astype(np.float32)
    wts = np.concatenate([w["c1_W1"], w2p, w["c2_W1"], w["c2_W2"]],
                         axis=1).astype(np.float32)
    wcat = np.concatenate([w["cls_W"], w["sim_W"], w["hom_W"], w["ent_W"]],
                          axis=1).astype(np.float32)
    bcat = np.concatenate([w["cls_b"], w["sim_b"], w["hom_b"], w["ent_b"]]
                          ).astype(np.float32)[None, :]
    return wts, wcat, bcat, vecs, t2.astype(np.float32)[None, :]


def _make_in_maps(cfg, x, arrs, weights):
    idx1, dl1, idx2, dl2 = arrs
    npc, npad = cfg["npc"], cfg["npad"]
    wts, wcat, bcat, vecs, trow = _fold_weights(weights)
    x_bf = np.ascontiguousarray(x.astype(npbf))
    in_maps = []
    for c in range(cfg["n_cores"]):
        xt = np.zeros((P, npad), np.float32)
        xt[:, :npc] = x[c * npc:(c + 1) * npc].T
        in_maps.append({
            "x_bf": x_bf, "xT": xt,
            "idx0": idx1[c], "dl0": dl1[c],
            "idx1": idx2[c], "dl1": dl2[c],
            "wts": wts, "wcat": wcat, "bcat": bcat, "vecs": vecs,
            "trow": trow,
        })
    return in_maps


def _run(x, edge_index, weights, n_cores=N_CORES, split=SPLIT):
    x = np.ascontiguousarray(np.asarray(x, np.float32))
    key = (x.shape, hash(np.asarray(edge_index).tobytes()), n_cores, split)
    if key not in _CACHE:
        cfg, plans, arrs = _prepare(x, edge_index, weights, n_cores, split)
        nc = _build_program(cfg, plans)
        _CACHE.clear()
        _CACHE[key] = (cfg, nc, arrs)
    cfg, nc, arrs = _CACHE[key]
    in_maps = _make_in_maps(cfg, x, arrs, weights)
    res = bass_utils.run_bass_kernel_spmd(
        nc, in_maps, core_ids=list(range(n_cores)))
    npc = cfg["npc"]
    full = np.concatenate(
        [res.results[c]["out_all"][:npc] for c in range(n_cores)], axis=0)
    return (np.ascontiguousarray(full[:, 0:40]),
            np.ascontiguousarray(full[:, 40:80]),
            np.ascontiguousarray(full[:, 80]),
            np.ascontiguousarray(full[:, 81]))


def bench_ns(inputs, iters=10):
    """Wall-clock repeated executions of the cached compiled NEFF on the 8
    cores (inputs device-resident, async dispatch pipelined). Under axon this
    is dominated by per-call tunnel overhead (~10-30 ms) and input
    re-shipping, so it is only an upper bound on NEFF exec time."""
    import time

    import jax
    from jax.experimental.shard_map import shard_map
    from jax.sharding import Mesh, PartitionSpec

    from concourse import bass2jax
    from concourse.bass2jax import _bass_exec_p, partition_id_tensor

    x = np.ascontiguousarray(np.asarray(inputs["x"], np.float32))
    edge_index = np.asarray(inputs["edge_index"], np.int64)
    weights = {k: np.asarray(v, np.float32) for k, v in inputs.items()
               if k not in ("x", "edge_index")}
    key = (x.shape, hash(np.asarray(edge_index).tobytes()), N_CORES, SPLIT)
    if key not in _CACHE:
        _run(x, edge_index, weights)
    cfg, nc, arrs = _CACHE[key]
    in_maps = _make_in_maps(cfg, x, arrs, weights)

    bass2jax.install_neuronx_cc_hook()
    in_names, out_names, out_avals, zero_outs = [], [], [], []
    partition_name = (nc.partition_id_tensor.name
                      if nc.partition_id_tensor else None)
    for alloc in nc.m.functions[0].allocations:
        if not isinstance(alloc, mybir.MemoryLocationSet):
            continue
        name = alloc.memorylocations[0].name
        if alloc.kind == "ExternalInput":
            if name != partition_name:
                in_names.append(name)
        elif alloc.kind == "ExternalOutput":
            shape = tuple(alloc.tensor_shape)
            dtype = mybir.dt.np(alloc.dtype)
            out_names.append(name)
            out_avals.append(jax.core.ShapedArray(shape, dtype))
            zero_outs.append(np.zeros(shape, dtype))
    n_params = len(in_names)
    all_in_names = list(in_names) + list(out_names)
    if partition_name is not None:
        all_in_names.append(partition_name)

    def _body(*args):
        operands = list(args)
        if partition_name is not None:
            operands.append(partition_id_tensor())
        outs = _bass_exec_p.bind(
            *operands,
            out_avals=tuple(out_avals),
            in_names=tuple(all_in_names),
            out_names=tuple(out_names),
            lowering_input_output_aliases=(),
            sim_require_finite=True,
            sim_require_nnan=True,
            nc=nc,
        )
        return tuple(outs)

    devices = jax.devices()[:N_CORES]
    mesh = Mesh(np.asarray(devices), ("core",))
    n_outs = len(out_avals)
    sharded = jax.jit(
        shard_map(_body, mesh=mesh,
                  in_specs=(PartitionSpec("core"),) * (n_params + n_outs),
                  out_specs=(PartitionSpec("core"),) * n_outs,
                  check_rep=False),
        keep_unused=True,
    )
    concat_in = [
        np.concatenate([np.asarray(in_maps[c][nm])
                        for c in range(N_CORES)], axis=0)
        for nm in in_names
    ]
    concat_zeros = [
        np.zeros((N_CORES * z.shape[0], *z.shape[1:]), z.dtype)
        for z in zero_outs
    ]
    args = [jax.device_put(a) for a in concat_in + concat_zeros]
    out = sharded(*args)
    jax.block_until_ready(out)
    t0 = time.perf_counter()
    outs = [sharded(*args) for _ in range(iters)]
    jax.block_until_ready(outs)
    dt = (time.perf_counter() - t0) / iters
    return dt * 1e9


def kernel(x, edge_index,
           c1_W1, c1_b1, c1_g1, c1_be1, c1_W2, c1_b2,
           c2_W1, c2_b1, c2_g1, c2_be1, c2_W2, c2_b2,
           bn_g, bn_b,
           cls_W, cls_b, sim_W, sim_b, hom_W, hom_b, ent_W, ent_b):
    weights = dict(
        c1_W1=np.asarray(c1_W1, np.float32), c1_b1=np.asarray(c1_b1, np.float32),
        c1_g1=np.asarray(c1_g1, np.float32), c1_be1=np.asarray(c1_be1, np.float32),
        c1_W2=np.asarray(c1_W2, np.float32), c1_b2=np.asarray(c1_b2, np.float32),
        c2_W1=np.asarray(c2_W1, np.float32), c2_b1=np.asarray(c2_b1, np.float32),
        c2_g1=np.asarray(c2_g1, np.float32), c2_be1=np.asarray(c2_be1, np.float32),
        c2_W2=np.asarray(c2_W2, np.float32), c2_b2=np.asarray(c2_b2, np.float32),
        bn_g=np.asarray(bn_g, np.float32), bn_b=np.asarray(bn_b, np.float32),
        cls_W=np.asarray(cls_W, np.float32), cls_b=np.asarray(cls_b, np.float32),
        sim_W=np.asarray(sim_W, np.float32), sim_b=np.asarray(sim_b, np.float32),
        hom_W=np.asarray(hom_W, np.float32), hom_b=np.asarray(hom_b, np.float32),
        ent_W=np.asarray(ent_W, np.float32), ent_b=np.asarray(ent_b, np.float32),
    )
    return _run(np.asarray(x, np.float32), np.asarray(edge_index, np.int64),
                weights)
